# revision 18
# baseline (speedup 1.0000x reference)
"""Single-head causal attention (B=4, T=2048, D=1024, HS=64) on 8 TRN2 cores.

Sharding: 2 cores per batch element (batch = core % 4, role = core // 4):
  role 0: blocks {0,1,2,12,13,14,15} (64 causal 128x128 tiles)
  role 1: blocks {3,4..11}           (72 tiles; block 3 offsets role 0's
                                      extra kv chunk)

Projections: bf16 hi/lo 3-group matmuls (xh*wh + xl*wh + xh*wl) in fp32
PSUM - x ships as interleaved hi/lo pairs (4B/elem, 2KB DMA rows).
Scores: k^T, q^T stored as float32r (ACT rounds on copy); ONE f32r
matmul per 512-chunk (no on-device hi/lo re-splitting of q/k).
Softmax: chunked row-max (DVE) + exp on ACT (scale=8, bias=-8*max),
E bf16; E^T via PE transposes (4 per PSUM tile) with PSUM->SBUF copies
alternating scalar/vector; out^T = v^T @ E^T accumulated in PSUM.

x streams strictly chunk-by-chunk (16 transfers/chunk on the sync
queue) in per-role order (role 0: [g0,g3,g1,g2], role 1: [g0,g1,g2,-]);
role 0 loads its 4th chunk inside its branch and pre-runs block 12's
score chunks on the first three kv chunks while it streams. Attention
pools are opened per-branch (shared pools across unequal If branches
corrupt tile buffer rotation).

Device emits unnormalized out^T [64,1152] + Z [128,9]; host divides and
transposes (fp32).
"""

import numpy as np

N_CORES = 8
B, T, D, HS = 4, 2048, 1024, 64
P = 128
NT = T // P
ND = D // P
NCH = 4
SCALE = 8.0        # sqrt(HS)
NEG = -1.0e30
NSLOT = 9

ROLE_BLOCKS = [
    [0, 1, 2, 12, 13, 14, 15],
    [3, 4, 5, 6, 7, 8, 9, 10, 11],
]
ROLE_XORDER = [[0, 3, 1, 2], [0, 1, 2, 3]]  # role 1 slot 3 unused

_COMPILED = None


def _build():
    import concourse.bass as bass
    import concourse.tile as tile
    from concourse import bacc, mybir

    f32 = mybir.dt.float32
    f32r = mybir.dt.float32r
    bf16 = mybir.dt.bfloat16
    EXP = mybir.ActivationFunctionType.Exp
    AX = mybir.AxisListType.X

    nc = bacc.Bacc("TRN2", target_bir_lowering=False, debug=False,
                   num_devices=N_CORES)

    xhl_d = nc.dram_tensor("xhl", [D, 4, 1024], bf16,
                           kind="ExternalInput").ap()
    wkvh_d = nc.dram_tensor("wkvh", [P, ND * P], bf16, kind="ExternalInput").ap()
    wkvl_d = nc.dram_tensor("wkvl", [P, ND * P], bf16, kind="ExternalInput").ap()
    wqh_d = nc.dram_tensor("wqh", [P, ND * HS], bf16, kind="ExternalInput").ap()
    wql_d = nc.dram_tensor("wql", [P, ND * HS], bf16, kind="ExternalInput").ap()
    identb_d = nc.dram_tensor("identb", [P, P], bf16, kind="ExternalInput").ap()
    mask_d = nc.dram_tensor("mask", [P, P], f32, kind="ExternalInput").ap()
    outT_d = nc.dram_tensor("outT", [HS, NSLOT * P], f32,
                            kind="ExternalOutput").ap()
    z_d = nc.dram_tensor("z", [P, NSLOT], f32, kind="ExternalOutput").ap()

    with tile.TileContext(nc) as tc:
        with tc.tile_pool(name="consts", bufs=1) as consts, \
             tc.tile_pool(name="big", bufs=1) as big:
            identb = consts.tile([P, P], bf16)
            mask = consts.tile([P, P], f32)
            wkvh = consts.tile([P, ND, P], bf16)
            wkvl = consts.tile([P, ND, P], bf16)
            wqh = consts.tile([P, ND, HS], bf16)
            wql = consts.tile([P, ND, HS], bf16)
            nc.scalar.dma_start(wkvh[:], wkvh_d.rearrange("p (a h) -> p a h", a=ND))
            nc.scalar.dma_start(wkvl[:], wkvl_d.rearrange("p (a h) -> p a h", a=ND))
            nc.scalar.dma_start(identb[:], identb_d[:])
            nc.scalar.dma_start(wqh[:], wqh_d.rearrange("p (a h) -> p a h", a=ND))
            nc.scalar.dma_start(wql[:], wql_d.rearrange("p (a h) -> p a h", a=ND))
            nc.scalar.dma_start(mask[:], mask_d[:])

            # x^T hi/lo slots, streamed strictly in slot order
            xc = [big.tile([P, ND, 1024], bf16, name=f"xc{s}", tag=f"xc{s}")
                  for s in range(4)]

            def load_x(s, nway=16):
                px = P // (nway // ND)
                for dt in range(ND):
                    for ph in range(nway // ND):
                        nc.sync.dma_start(
                            xc[s][ph * px:(ph + 1) * px, dt, :],
                            xhl_d[dt * P + ph * px:dt * P + (ph + 1) * px,
                                  s, :])

            load_x(0, nway=32)
            load_x(1)
            load_x(2)

            KHL = big.tile([P, T], bf16)    # rows 0:64 k_lo, 64:128 k_hi
            vTb = big.tile([HS, NCH, 512], bf16)
            vn = big.tile([P, NT, HS], bf16)
            qhh = big.tile([P, 2, 512], bf16)   # 0:64 = 64:128 = q_hi
            qlz = big.tile([P, 2, 512], bf16)   # 0:64 = 0, 64:128 = q_lo
            qxh = big.tile([P, P], bf16)
            qxl = big.tile([P, P], bf16)
            avs_all = big.tile([HS, NSLOT * P], f32)
            z_all = big.tile([P, NSLOT], f32)
            nc.vector.memset(qlz[0:HS, :, :], 0.0)
            nc.vector.memset(qxl[0:HS, :], 0.0)

            NGRP = [(0, 0), (0, 512), (1, 0)]   # (use wl?, x col offset)

            def kv_chunk(s, g):
                cs = slice(g * 512, (g + 1) * 512)
                ps = apools["spool"].tile([P, 512], f32, tag="S",
                                          name="pskv")
                i, n = 0, 3 * ND
                for wl, xoff in NGRP:
                    w_t = wkvl if wl else wkvh
                    for dt in range(ND):
                        nc.tensor.matmul(
                            ps[:], lhsT=w_t[:, dt, :],
                            rhs=xc[s][:, dt, xoff:xoff + 512],
                            start=(i == 0), stop=(i == n - 1))
                        i += 1
                nc.scalar.copy(KHL[HS:P, cs], ps[HS:P, :])
                nc.scalar.copy(vTb[:, g, :], ps[0:HS, :])
                kl = apools["ets"].tile([P, 512], bf16, tag="ets",
                                        name="kl")
                nc.vector.tensor_sub(kl[HS:P, :], ps[HS:P, :], KHL[HS:P, cs])
                nc.gpsimd.dma_start(KHL[0:HS, cs], kl[HS:P, :])
                for tt in range(4):
                    vpf = apools["spool"].tile([P, 512], f32, tag="S",
                                               name="vpf")
                    vp = vpf[:, 0:32].bitcast(bf16)
                    nc.tensor.transpose(
                        vp, vTb[:, g, tt * P:(tt + 1) * P],
                        identb[0:HS, 0:HS])
                    if tt % 2:
                        nc.scalar.copy(vn[:, g * 4 + tt, :], vp)
                    else:
                        nc.vector.tensor_copy(vn[:, g * 4 + tt, :], vp)

            def q_chunk(s, qc):
                psf = apools["spool"].tile([P, 512], f32, tag="S",
                                           name="psq")
                ps = psf[0:HS, :]
                i, n = 0, 3 * ND
                for wl, xoff in NGRP:
                    w_t = wql if wl else wqh
                    for dt in range(ND):
                        nc.tensor.matmul(
                            ps, lhsT=w_t[:, dt, :],
                            rhs=xc[s][:, dt, xoff:xoff + 512],
                            start=(i == 0), stop=(i == n - 1))
                        i += 1
                nc.scalar.copy(qhh[0:HS, qc, :], ps)
                qt = apools["ets"].tile([P, 512], bf16, tag="ets",
                                        name="qt")
                nc.vector.tensor_sub(qt[0:HS, :], ps, qhh[0:HS, qc, :])
                nc.gpsimd.dma_start(qhh[HS:P, qc, :], qhh[0:HS, qc, :])
                nc.gpsimd.dma_start(qlz[HS:P, qc, :], qt[0:HS, :])

            def q_extra(s, off):
                psf = apools["spool"].tile([P, 512], f32, tag="S",
                                           name="psx")
                ps = psf[0:HS, 0:P]
                i, n = 0, 3 * ND
                for wl, xoff in NGRP:
                    w_t = wql if wl else wqh
                    for dt in range(ND):
                        nc.tensor.matmul(
                            ps, lhsT=w_t[:, dt, :],
                            rhs=xc[s][:, dt, xoff + off:xoff + off + P],
                            start=(i == 0), stop=(i == n - 1))
                        i += 1
                nc.scalar.copy(qxh[0:HS, :], ps)
                qt = apools["ets"].tile([P, 512], bf16, tag="ets",
                                        name="qtx")
                nc.vector.tensor_sub(qt[0:HS, 0:P], ps, qxh[0:HS, :])
                nc.gpsimd.dma_start(qxh[HS:P, :], qxh[0:HS, :])
                nc.gpsimd.dma_start(qxl[HS:P, :], qt[0:HS, 0:P])

            apools = {}

            def open_attn_pools():
                cms = [tc.tile_pool(name="spool", bufs=5, space="PSUM"),
                       tc.tile_pool(name="etp", bufs=2, space="PSUM"),
                       tc.tile_pool(name="avp", bufs=1, space="PSUM"),
                       tc.tile_pool(name="epool", bufs=3),
                       tc.tile_pool(name="ets", bufs=4),
                       tc.tile_pool(name="small", bufs=4)]
                for k, cm in zip(("spool", "etp", "avp", "epool", "ets",
                                  "small"), cms):
                    apools[k] = cm.__enter__()
                return cms

            def close_attn_pools(cms):
                for cm in reversed(cms):
                    cm.__exit__(None, None, None)

            def begin_block(j):
                nch = (128 * (j + 1) + 511) // 512
                mja = apools["small"].tile([P, nch], f32, tag="mja")
                return {"mja": mja, "sps": [], "i": 0}

            def s_chunk(j, kc, q_ap, st):
                qh_ap, ql_ap = q_ap
                L = 128 * (j + 1)
                w = min(512, L - kc * 512)
                sp = apools["spool"].tile([P, 512], f32, tag="S")
                rhs = KHL[:, kc * 512:kc * 512 + w]
                nc.tensor.matmul(sp[:, 0:w], lhsT=qh_ap, rhs=rhs,
                                 start=True, stop=False)
                nc.tensor.matmul(sp[:, 0:w], lhsT=ql_ap, rhs=rhs,
                                 start=False, stop=True)
                if kc == j // 4:  # diagonal chunk
                    nc.vector.tensor_add(
                        sp[:, w - P:w], sp[:, w - P:w], mask[:])
                nc.vector.reduce_max(st["mja"][:, st["i"]:st["i"] + 1],
                                     sp[:, 0:w], axis=AX)
                st["i"] += 1
                st["sps"].append((kc, sp, w))

            def finish_block(j, slot, st):
                L = 128 * (j + 1)
                sps = st["sps"]
                nch = len(sps)
                nm8 = apools["small"].tile([P, 1], f32, tag="nm8")
                if nch == 1:
                    nc.vector.tensor_scalar_mul(nm8[:], st["mja"][:], -SCALE)
                else:
                    m = apools["small"].tile([P, 1], f32, tag="m")
                    nc.vector.reduce_max(m[:], st["mja"][:], axis=AX)
                    nc.vector.tensor_scalar_mul(nm8[:], m[:], -SCALE)

                E = apools["epool"].tile([P, L], bf16, tag="E")
                zc = apools["small"].tile([P, nch], f32, tag="zc")
                for i, (kc, sp, w) in enumerate(sps):
                    nc.scalar.activation(
                        E[:, kc * 512:kc * 512 + w], sp[:, 0:w], EXP,
                        bias=nm8[:], scale=SCALE,
                        accum_out=zc[:, i:i + 1])
                if nch == 1:
                    nc.vector.tensor_copy(z_all[:, slot:slot + 1], zc[:, 0:1])
                else:
                    nc.vector.reduce_sum(z_all[:, slot:slot + 1], zc[:],
                                         axis=AX)

                av = apools["avp"].tile([HS, P], f32, tag="av")
                nkt = L // P
                kt = 0
                gi = 0
                while kt < nkt:
                    gn = min(4, nkt - kt)
                    ep = apools["etp"].tile([P, 512], bf16, tag="ep")
                    for u in range(gn):
                        nc.tensor.transpose(
                            ep[:, u * P:(u + 1) * P],
                            E[:, (kt + u) * P:(kt + u + 1) * P],
                            identb[:])
                    es = apools["ets"].tile([P, 512], bf16, tag="ets")
                    if gi % 3 == 0:
                        nc.scalar.copy(es[:, 0:gn * P], ep[:, 0:gn * P])
                    else:
                        nc.vector.tensor_copy(es[:, 0:gn * P],
                                              ep[:, 0:gn * P])
                    for u in range(gn):
                        nc.tensor.matmul(
                            av[:], lhsT=vn[:, kt + u, :],
                            rhs=es[:, u * P:(u + 1) * P],
                            start=(kt + u == 0), stop=(kt + u == nkt - 1),
                            skip_group_check=True)
                    kt += gn
                    gi += 1
                nc.vector.tensor_copy(
                    avs_all[:, slot * P:(slot + 1) * P], av[:])

            def emit_block(j, slot, q_ap):
                nch = (128 * (j + 1) + 511) // 512
                st = begin_block(j)
                for kc in range(nch):
                    s_chunk(j, kc, q_ap, st)
                finish_block(j, slot, st)

            def role0():
                cms = open_attn_pools()
                load_x(3)                       # global g2, arrives last
                kv_chunk(0, 0)
                q_chunk(0, 0)                   # q rows 0:512
                q0 = lambda jj: (qhh[:, 0, 128 * jj:128 * jj + P],
                                 qlz[:, 0, 128 * jj:128 * jj + P])
                emit_block(0, 0, q0(0))
                emit_block(1, 1, q0(1))
                q_chunk(1, 1)                   # q rows 1536:2048 (g3)
                kv_chunk(1, 3)                  # g3
                emit_block(2, 2, q0(2))
                kv_chunk(2, 1)                  # g1
                q1 = lambda jj: (
                    qhh[:, 1, 128 * (jj - 12):128 * (jj - 12) + P],
                    qlz[:, 1, 128 * (jj - 12):128 * (jj - 12) + P])
                st12 = begin_block(12)
                for kc in (0, 3, 1):
                    s_chunk(12, kc, q1(12), st12)
                kv_chunk(3, 2)                  # g2
                s_chunk(12, 2, q1(12), st12)
                finish_block(12, 3, st12)
                emit_block(13, 4, q1(13))
                emit_block(14, 5, q1(14))
                emit_block(15, 6, q1(15))
                close_attn_pools(cms)

            def role1():
                cms = open_attn_pools()
                kv_chunk(0, 0)
                q_extra(0, 384)                 # block 3 q rows (x g0)
                emit_block(3, 0, (qxh[:], qxl[:]))
                q_chunk(1, 0)                   # q rows 512:1024  (g1)
                kv_chunk(1, 1)                  # g1
                qa = lambda jj: (
                    qhh[:, 0, 128 * (jj - 4):128 * (jj - 4) + P],
                    qlz[:, 0, 128 * (jj - 4):128 * (jj - 4) + P])
                qb = lambda jj: (
                    qhh[:, 1, 128 * (jj - 8):128 * (jj - 8) + P],
                    qlz[:, 1, 128 * (jj - 8):128 * (jj - 8) + P])
                emit_block(4, 1, qa(4))
                emit_block(5, 2, qa(5))
                q_chunk(2, 1)                   # q rows 1024:1536 (g2)
                kv_chunk(2, 2)                  # g2
                emit_block(6, 3, qa(6))
                emit_block(7, 4, qa(7))
                for slot, j in ((5, 8), (6, 9), (7, 10), (8, 11)):
                    emit_block(j, slot, qb(j))
                close_attn_pools(cms)

            pid = nc.partition_id()
            with tc.If(pid < 4) as cmp:
                role0()
            with cmp.Else():
                role1()

            nc.sync.dma_start(outT_d[:], avs_all[:])
            nc.sync.dma_start(z_d[:], z_all[:])

    nc.compile()
    return nc


def _get_program():
    global _COMPILED
    if _COMPILED is None:
        _COMPILED = _build()
    return _COMPILED


def _install_ntff_hook():
    import sys, types
    if "antenv.axon_hooks" in sys.modules:
        return
    try:
        from trn_agent_boot.trn_boot import _ntff_profile_via_ctypes
        hook = _ntff_profile_via_ctypes("/opt/axon/libaxon_pjrt.so")
        mod = types.ModuleType("antenv.axon_hooks")
        mod.get_axon_ntff_profile_hook = lambda: hook
        mod.set_axon_ntff_profile_hook = lambda h: None
        import antenv
        sys.modules["antenv.axon_hooks"] = mod
        antenv.axon_hooks = mod
    except Exception:
        pass


def _split_pair(a):
    import ml_dtypes
    hi = a.astype(ml_dtypes.bfloat16)
    lo = (a - hi.astype(np.float32)).astype(ml_dtypes.bfloat16)
    return hi, lo


def _host_prep(inputs):
    import ml_dtypes
    x = np.asarray(inputs["x"], dtype=np.float32)
    wq = np.asarray(inputs["Wq"], dtype=np.float32)
    wk = np.asarray(inputs["Wk"], dtype=np.float32)
    wv = np.asarray(inputs["Wv"], dtype=np.float32)

    xt = np.transpose(x, (0, 2, 1)).reshape(B, D, NCH, 512)
    hi, lo = _split_pair(np.ascontiguousarray(xt))
    xhl = np.concatenate([hi, lo], axis=3)         # [B, D, NCH, 1024]

    def _wprep(wt):
        m = wt.shape[1]
        return np.ascontiguousarray(
            wt.reshape(ND, P, m).transpose(1, 0, 2).reshape(P, ND * m))

    wkvh, wkvl = _split_pair(_wprep(np.concatenate([wv, wk], axis=0).T))
    wqh, wql = _split_pair(_wprep(wq.T))
    identb = np.eye(P, dtype=ml_dtypes.bfloat16)
    r = np.arange(P)
    mask = np.where(r[None, :] <= r[:, None], 0.0, NEG).astype(np.float32)

    shared = {"wkvh": wkvh, "wkvl": wkvl, "wqh": wqh, "wql": wql,
              "identb": identb, "mask": mask}
    in_maps = []
    for c in range(N_CORES):
        b, role = c % B, c // B
        m = dict(shared)
        m["xhl"] = np.ascontiguousarray(xhl[b][:, ROLE_XORDER[role], :])
        in_maps.append(m)
    return in_maps


def _run(inputs, trace=False):
    from concourse.bass_utils import run_bass_kernel_spmd

    if trace:
        _install_ntff_hook()
    nc = _get_program()
    in_maps = _host_prep(inputs)
    res = run_bass_kernel_spmd(nc, in_maps, list(range(N_CORES)), trace=trace)

    out = np.empty((B, T, HS), dtype=np.float32)
    for c in range(N_CORES):
        b, role = c % B, c // B
        avT = res.results[c]["outT"]
        z = res.results[c]["z"]
        for slot, j in enumerate(ROLE_BLOCKS[role]):
            blk = avT[:, P * slot:P * (slot + 1)].T
            out[b, P * j:P * (j + 1)] = blk / z[:, slot:slot + 1]
    return out, res


def kernel(**inputs):
    out, _ = _run(inputs, trace=False)
    return out


# revision 21
# speedup vs baseline: 1.0392x; 1.0392x over previous
"""Single-head causal attention (B=4, T=2048, D=1024, HS=64) on 8 TRN2 cores.

Sharding: 2 cores per batch element (batch = core % 4, role = core // 4):
  role 0: blocks {0,1,2,12,13,14,15} (64 causal 128x128 tiles)
  role 1: blocks {3,4..11}           (72 tiles; block 3 offsets role 0's
                                      extra kv chunk)

Projections: bf16 hi/lo 3-group matmuls (xh*wh + xl*wh + xh*wl) in fp32
PSUM - x ships as interleaved hi/lo pairs (4B/elem, 2KB DMA rows).
Scores: k^T, q^T stored as float32r (ACT rounds on copy); ONE f32r
matmul per 512-chunk (no on-device hi/lo re-splitting of q/k).
Softmax: chunked row-max (DVE) + exp on ACT (scale=8, bias=-8*max),
E bf16; E^T via PE transposes (4 per PSUM tile) with PSUM->SBUF copies
alternating scalar/vector; out^T = v^T @ E^T accumulated in PSUM.

x streams strictly chunk-by-chunk (16 transfers/chunk on the sync
queue) in per-role order (role 0: [g0,g3,g1,g2], role 1: [g0,g1,g2,-]);
role 0 loads its 4th chunk inside its branch and pre-runs block 12's
score chunks on the first three kv chunks while it streams. Attention
pools are opened per-branch (shared pools across unequal If branches
corrupt tile buffer rotation).

Device emits unnormalized out^T [64,1152] + Z [128,9]; host divides and
transposes (fp32).
"""

import numpy as np

N_CORES = 8
B, T, D, HS = 4, 2048, 1024, 64
P = 128
NT = T // P
ND = D // P
NCH = 4
SCALE = 8.0        # sqrt(HS)
NEG = -1.0e30
NSLOT = 9

ROLE_BLOCKS = [
    [0, 1, 2, 12, 13, 14, 15],
    [3, 4, 5, 6, 7, 8, 9, 10, 11],
]
ROLE_XORDER = [[0, 3, 1, 2], [0, 1, 2, 3]]  # role 1 slot 3 unused

_COMPILED = None


def _build():
    import concourse.bass as bass
    import concourse.tile as tile
    from concourse import bacc, mybir

    f32 = mybir.dt.float32
    f32r = mybir.dt.float32r
    bf16 = mybir.dt.bfloat16
    EXP = mybir.ActivationFunctionType.Exp
    AX = mybir.AxisListType.X

    nc = bacc.Bacc("TRN2", target_bir_lowering=False, debug=False,
                   num_devices=N_CORES)

    xhl_d = nc.dram_tensor("xhl", [D, 4, 1024], bf16,
                           kind="ExternalInput").ap()
    wkvh_d = nc.dram_tensor("wkvh", [P, ND * P], bf16, kind="ExternalInput").ap()
    wkvl_d = nc.dram_tensor("wkvl", [P, ND * P], bf16, kind="ExternalInput").ap()
    wqh_d = nc.dram_tensor("wqh", [P, ND * HS], bf16, kind="ExternalInput").ap()
    wql_d = nc.dram_tensor("wql", [P, ND * HS], bf16, kind="ExternalInput").ap()
    identb_d = nc.dram_tensor("identb", [P, P], bf16, kind="ExternalInput").ap()
    mask_d = nc.dram_tensor("mask", [P, P], f32, kind="ExternalInput").ap()
    outT_d = nc.dram_tensor("outT", [HS, NSLOT * P], f32,
                            kind="ExternalOutput").ap()
    z_d = nc.dram_tensor("z", [P, NSLOT], f32, kind="ExternalOutput").ap()

    with tile.TileContext(nc) as tc:
        with tc.tile_pool(name="consts", bufs=1) as consts, \
             tc.tile_pool(name="big", bufs=1) as big:
            identb = consts.tile([P, P], bf16)
            mask = consts.tile([P, P], f32)
            wkvh = consts.tile([P, ND, P], bf16)
            wkvl = consts.tile([P, ND, P], bf16)
            wqh = consts.tile([P, ND, HS], bf16)
            wql = consts.tile([P, ND, HS], bf16)
            for pp in range(4):
                rs = slice(32 * pp, 32 * (pp + 1))
                nc.scalar.dma_start(
                    wkvh[rs, :, :],
                    wkvh_d[rs, :].rearrange("p (a h) -> p a h", a=ND))
                nc.scalar.dma_start(
                    wkvl[rs, :, :],
                    wkvl_d[rs, :].rearrange("p (a h) -> p a h", a=ND))
            nc.scalar.dma_start(identb[:], identb_d[:])
            for pp in range(2):
                rs = slice(64 * pp, 64 * (pp + 1))
                nc.scalar.dma_start(
                    wqh[rs, :, :],
                    wqh_d[rs, :].rearrange("p (a h) -> p a h", a=ND))
                nc.scalar.dma_start(
                    wql[rs, :, :],
                    wql_d[rs, :].rearrange("p (a h) -> p a h", a=ND))
            nc.scalar.dma_start(mask[:], mask_d[:])

            # x^T hi/lo slots, streamed strictly in slot order
            xc = [big.tile([P, ND, 1024], bf16, name=f"xc{s}", tag=f"xc{s}")
                  for s in range(4)]

            def load_x(s, nway=16):
                px = P // (nway // ND)
                for dt in range(ND):
                    for ph in range(nway // ND):
                        nc.sync.dma_start(
                            xc[s][ph * px:(ph + 1) * px, dt, :],
                            xhl_d[dt * P + ph * px:dt * P + (ph + 1) * px,
                                  s, :])

            load_x(0, nway=32)
            load_x(1)
            load_x(2)

            KHL = big.tile([P, T], bf16)    # rows 0:64 k_lo, 64:128 k_hi
            vTb = big.tile([HS, NCH, 512], bf16)
            vn = big.tile([P, NT, HS], bf16)
            qhh = big.tile([P, 2, 512], bf16)   # 0:64 = 64:128 = q_hi
            qlz = big.tile([P, 2, 512], bf16)   # 0:64 = 0, 64:128 = q_lo
            qxh = big.tile([P, P], bf16)
            qxl = big.tile([P, P], bf16)
            avs_all = big.tile([HS, NSLOT * P], f32)
            z_all = big.tile([P, NSLOT], f32)
            nc.vector.memset(qlz[0:HS, :, :], 0.0)
            nc.vector.memset(qxl[0:HS, :], 0.0)

            NGRP = [(0, 0), (0, 512), (1, 0)]   # (use wl?, x col offset)

            def kv_chunk(s, g):
                cs = slice(g * 512, (g + 1) * 512)
                ps = apools["spool"].tile([P, 512], f32, tag="S",
                                          name="pskv")
                i, n = 0, 3 * ND
                for wl, xoff in NGRP:
                    w_t = wkvl if wl else wkvh
                    for dt in range(ND):
                        nc.tensor.matmul(
                            ps[:], lhsT=w_t[:, dt, :],
                            rhs=xc[s][:, dt, xoff:xoff + 512],
                            start=(i == 0), stop=(i == n - 1))
                        i += 1
                nc.scalar.copy(KHL[HS:P, cs], ps[HS:P, :])
                nc.scalar.copy(vTb[:, g, :], ps[0:HS, :])
                kl = apools["ets"].tile([P, 512], bf16, tag="ets",
                                        name="kl")
                nc.vector.tensor_sub(kl[HS:P, :], ps[HS:P, :], KHL[HS:P, cs])
                nc.gpsimd.dma_start(KHL[0:HS, cs], kl[HS:P, :])
                for tt in range(4):
                    vpf = apools["etp"].tile([P, 512], bf16, tag="ep",
                                             name="vpf")
                    vp = vpf[:, 0:HS]
                    nc.tensor.transpose(
                        vp, vTb[:, g, tt * P:(tt + 1) * P],
                        identb[0:HS, 0:HS])
                    if tt % 2:
                        nc.scalar.copy(vn[:, g * 4 + tt, :], vp)
                    else:
                        nc.vector.tensor_copy(vn[:, g * 4 + tt, :], vp)

            def q_chunk(s, qc):
                psf = apools["spool"].tile([P, 512], f32, tag="S",
                                           name="psq")
                ps = psf[0:HS, :]
                i, n = 0, 3 * ND
                for wl, xoff in NGRP:
                    w_t = wql if wl else wqh
                    for dt in range(ND):
                        nc.tensor.matmul(
                            ps, lhsT=w_t[:, dt, :],
                            rhs=xc[s][:, dt, xoff:xoff + 512],
                            start=(i == 0), stop=(i == n - 1))
                        i += 1
                nc.scalar.copy(qhh[0:HS, qc, :], ps)
                qt = apools["ets"].tile([P, 512], bf16, tag="ets",
                                        name="qt")
                nc.vector.tensor_sub(qt[0:HS, :], ps, qhh[0:HS, qc, :])
                nc.gpsimd.dma_start(qhh[HS:P, qc, :], qhh[0:HS, qc, :])
                nc.gpsimd.dma_start(qlz[HS:P, qc, :], qt[0:HS, :])

            def q_extra(s, off):
                psf = apools["spool"].tile([P, 512], f32, tag="S",
                                           name="psx")
                ps = psf[0:HS, 0:P]
                i, n = 0, 3 * ND
                for wl, xoff in NGRP:
                    w_t = wql if wl else wqh
                    for dt in range(ND):
                        nc.tensor.matmul(
                            ps, lhsT=w_t[:, dt, :],
                            rhs=xc[s][:, dt, xoff + off:xoff + off + P],
                            start=(i == 0), stop=(i == n - 1))
                        i += 1
                nc.scalar.copy(qxh[0:HS, :], ps)
                qt = apools["ets"].tile([P, 512], bf16, tag="ets",
                                        name="qtx")
                nc.vector.tensor_sub(qt[0:HS, 0:P], ps, qxh[0:HS, :])
                nc.gpsimd.dma_start(qxh[HS:P, :], qxh[0:HS, :])
                nc.gpsimd.dma_start(qxl[HS:P, :], qt[0:HS, 0:P])

            apools = {}

            def open_attn_pools():
                cms = [tc.tile_pool(name="spool", bufs=5, space="PSUM"),
                       tc.tile_pool(name="etp", bufs=2, space="PSUM"),
                       tc.tile_pool(name="avp", bufs=1, space="PSUM"),
                       tc.tile_pool(name="epool", bufs=3),
                       tc.tile_pool(name="ets", bufs=4),
                       tc.tile_pool(name="small", bufs=4)]
                for k, cm in zip(("spool", "etp", "avp", "epool", "ets",
                                  "small"), cms):
                    apools[k] = cm.__enter__()
                av2t = apools["avp"].tile([HS, 2, P], f32, tag="av2",
                                          name="av2t")
                apools["av2t"] = av2t
                return cms

            def close_attn_pools(cms):
                for cm in reversed(cms):
                    cm.__exit__(None, None, None)

            def begin_block(j):
                nch = (128 * (j + 1) + 511) // 512
                mja = apools["small"].tile([P, nch], f32, tag="mja")
                return {"mja": mja, "sps": [], "i": 0,
                        "av2": apools["av2t"]}

            def s_chunk(j, kc, q_ap, st):
                qh_ap, ql_ap = q_ap
                L = 128 * (j + 1)
                w = min(512, L - kc * 512)
                sp = apools["spool"].tile([P, 512], f32, tag="S")
                rhs = KHL[:, kc * 512:kc * 512 + w]
                nc.tensor.matmul(sp[:, 0:w], lhsT=qh_ap, rhs=rhs,
                                 start=True, stop=False)
                nc.tensor.matmul(sp[:, 0:w], lhsT=ql_ap, rhs=rhs,
                                 start=False, stop=True)
                if kc == j // 4:  # diagonal chunk
                    nc.vector.tensor_add(
                        sp[:, w - P:w], sp[:, w - P:w], mask[:])
                nc.vector.reduce_max(st["mja"][:, st["i"]:st["i"] + 1],
                                     sp[:, 0:w], axis=AX)
                st["i"] += 1
                st["sps"].append((kc, sp, w))

            def finish_block(j, slot, st):
                L = 128 * (j + 1)
                sps = st["sps"]
                nch = len(sps)
                nm8 = apools["small"].tile([P, 1], f32, tag="nm8")
                if nch == 1:
                    nc.vector.tensor_scalar_mul(nm8[:], st["mja"][:], -SCALE)
                else:
                    m = apools["small"].tile([P, 1], f32, tag="m")
                    nc.vector.reduce_max(m[:], st["mja"][:], axis=AX)
                    nc.vector.tensor_scalar_mul(nm8[:], m[:], -SCALE)

                E = apools["epool"].tile([P, L], bf16, tag="E")
                zc = apools["small"].tile([P, nch], f32, tag="zc")
                for i, (kc, sp, w) in enumerate(sps):
                    nc.scalar.activation(
                        E[:, kc * 512:kc * 512 + w], sp[:, 0:w], EXP,
                        bias=nm8[:], scale=SCALE,
                        accum_out=zc[:, i:i + 1])
                if nch == 1:
                    nc.vector.tensor_copy(z_all[:, slot:slot + 1], zc[:, 0:1])
                else:
                    nc.vector.reduce_sum(z_all[:, slot:slot + 1], zc[:],
                                         axis=AX)

                av = st["av2"][:, slot % 2, :]
                nkt = L // P
                kt = 0
                gi = 0
                while kt < nkt:
                    gn = min(4, nkt - kt)
                    ep = apools["etp"].tile([P, 512], bf16, tag="ep")
                    for u in range(gn):
                        nc.tensor.transpose(
                            ep[:, u * P:(u + 1) * P],
                            E[:, (kt + u) * P:(kt + u + 1) * P],
                            identb[:])
                    es = apools["ets"].tile([P, 512], bf16, tag="ets")
                    if gi % 3 == 0:
                        nc.scalar.copy(es[:, 0:gn * P], ep[:, 0:gn * P])
                    else:
                        nc.vector.tensor_copy(es[:, 0:gn * P],
                                              ep[:, 0:gn * P])
                    for u in range(gn):
                        nc.tensor.matmul(
                            av[:], lhsT=vn[:, kt + u, :],
                            rhs=es[:, u * P:(u + 1) * P],
                            start=(kt + u == 0), stop=(kt + u == nkt - 1),
                            skip_group_check=True)
                    kt += gn
                    gi += 1
                nc.vector.tensor_copy(
                    avs_all[:, slot * P:(slot + 1) * P], av[:])

            def emit_block(j, slot, q_ap):
                nch = (128 * (j + 1) + 511) // 512
                st = begin_block(j)
                for kc in range(nch):
                    s_chunk(j, kc, q_ap, st)
                finish_block(j, slot, st)

            def role0():
                cms = open_attn_pools()
                load_x(3)                       # global g2, arrives last
                kv_chunk(0, 0)
                q_chunk(0, 0)                   # q rows 0:512
                q0 = lambda jj: (qhh[:, 0, 128 * jj:128 * jj + P],
                                 qlz[:, 0, 128 * jj:128 * jj + P])
                emit_block(0, 0, q0(0))
                emit_block(1, 1, q0(1))
                q_chunk(1, 1)                   # q rows 1536:2048 (g3)
                kv_chunk(1, 3)                  # g3
                emit_block(2, 2, q0(2))
                kv_chunk(2, 1)                  # g1
                q1 = lambda jj: (
                    qhh[:, 1, 128 * (jj - 12):128 * (jj - 12) + P],
                    qlz[:, 1, 128 * (jj - 12):128 * (jj - 12) + P])
                st12 = begin_block(12)
                for kc in (0, 3, 1):
                    s_chunk(12, kc, q1(12), st12)
                kv_chunk(3, 2)                  # g2
                s_chunk(12, 2, q1(12), st12)
                finish_block(12, 3, st12)
                emit_block(13, 4, q1(13))
                emit_block(14, 5, q1(14))
                emit_block(15, 6, q1(15))
                close_attn_pools(cms)

            def role1():
                cms = open_attn_pools()
                kv_chunk(0, 0)
                q_extra(0, 384)                 # block 3 q rows (x g0)
                emit_block(3, 0, (qxh[:], qxl[:]))
                q_chunk(1, 0)                   # q rows 512:1024  (g1)
                kv_chunk(1, 1)                  # g1
                qa = lambda jj: (
                    qhh[:, 0, 128 * (jj - 4):128 * (jj - 4) + P],
                    qlz[:, 0, 128 * (jj - 4):128 * (jj - 4) + P])
                qb = lambda jj: (
                    qhh[:, 1, 128 * (jj - 8):128 * (jj - 8) + P],
                    qlz[:, 1, 128 * (jj - 8):128 * (jj - 8) + P])
                emit_block(4, 1, qa(4))
                emit_block(5, 2, qa(5))
                q_chunk(2, 1)                   # q rows 1024:1536 (g2)
                kv_chunk(2, 2)                  # g2
                emit_block(6, 3, qa(6))
                emit_block(7, 4, qa(7))
                for slot, j in ((5, 8), (6, 9), (7, 10), (8, 11)):
                    emit_block(j, slot, qb(j))
                close_attn_pools(cms)

            pid = nc.partition_id()
            with tc.If(pid < 4) as cmp:
                role0()
            with cmp.Else():
                role1()

            nc.sync.dma_start(outT_d[:], avs_all[:])
            nc.sync.dma_start(z_d[:], z_all[:])

    nc.compile()
    return nc


def _get_program():
    global _COMPILED
    if _COMPILED is None:
        _COMPILED = _build()
    return _COMPILED


def _install_ntff_hook():
    import sys, types
    if "antenv.axon_hooks" in sys.modules:
        return
    try:
        from trn_agent_boot.trn_boot import _ntff_profile_via_ctypes
        hook = _ntff_profile_via_ctypes("/opt/axon/libaxon_pjrt.so")
        mod = types.ModuleType("antenv.axon_hooks")
        mod.get_axon_ntff_profile_hook = lambda: hook
        mod.set_axon_ntff_profile_hook = lambda h: None
        import antenv
        sys.modules["antenv.axon_hooks"] = mod
        antenv.axon_hooks = mod
    except Exception:
        pass


def _split_pair(a):
    import ml_dtypes
    hi = a.astype(ml_dtypes.bfloat16)
    lo = (a - hi.astype(np.float32)).astype(ml_dtypes.bfloat16)
    return hi, lo


def _host_prep(inputs):
    import ml_dtypes
    x = np.asarray(inputs["x"], dtype=np.float32)
    wq = np.asarray(inputs["Wq"], dtype=np.float32)
    wk = np.asarray(inputs["Wk"], dtype=np.float32)
    wv = np.asarray(inputs["Wv"], dtype=np.float32)

    xt = np.transpose(x, (0, 2, 1)).reshape(B, D, NCH, 512)
    hi, lo = _split_pair(np.ascontiguousarray(xt))
    xhl = np.concatenate([hi, lo], axis=3)         # [B, D, NCH, 1024]

    def _wprep(wt):
        m = wt.shape[1]
        return np.ascontiguousarray(
            wt.reshape(ND, P, m).transpose(1, 0, 2).reshape(P, ND * m))

    wkvh, wkvl = _split_pair(_wprep(np.concatenate([wv, wk], axis=0).T))
    wqh, wql = _split_pair(_wprep(wq.T))
    identb = np.eye(P, dtype=ml_dtypes.bfloat16)
    r = np.arange(P)
    mask = np.where(r[None, :] <= r[:, None], 0.0, NEG).astype(np.float32)

    shared = {"wkvh": wkvh, "wkvl": wkvl, "wqh": wqh, "wql": wql,
              "identb": identb, "mask": mask}
    in_maps = []
    for c in range(N_CORES):
        b, role = c % B, c // B
        m = dict(shared)
        m["xhl"] = np.ascontiguousarray(xhl[b][:, ROLE_XORDER[role], :])
        in_maps.append(m)
    return in_maps


def _run(inputs, trace=False):
    from concourse.bass_utils import run_bass_kernel_spmd

    if trace:
        _install_ntff_hook()
    nc = _get_program()
    in_maps = _host_prep(inputs)
    res = run_bass_kernel_spmd(nc, in_maps, list(range(N_CORES)), trace=trace)

    out = np.empty((B, T, HS), dtype=np.float32)
    for c in range(N_CORES):
        b, role = c % B, c // B
        avT = res.results[c]["outT"]
        z = res.results[c]["z"]
        for slot, j in enumerate(ROLE_BLOCKS[role]):
            blk = avT[:, P * slot:P * (slot + 1)].T
            out[b, P * j:P * (j + 1)] = blk / z[:, slot:slot + 1]
    return out, res


def kernel(**inputs):
    out, _ = _run(inputs, trace=False)
    return out
